# revision 5
# baseline (speedup 1.0000x reference)
"""Category-specific linear (MoE-style routed batched matmul) on 8 trn2 cores.

out[b, s, h] = sum_i x[b, s, i] * W[cat_ids[b], i, h] + bias[cat_ids[b], h]

Shapes (hardcoded): x (32, 512, 1024) f32, cat_ids (32,) int, W (16, 1024, 4096)
f32, b (16, 4096) f32 -> out (32, 512, 4096) f32.

Strategy: data-parallel over batch, 4 batches per core, with host-side routing
that always packs one same-category PAIR of batches plus two singles per core
(slot capacities [2, 1, 1] batches). With 32 batches over 16 categories there
are always >= (32 - 16)/2 = 8 disjoint same-category pairs, so this packing is
feasible for ANY cat_ids. Each core then loads only 3 weight matrices (24 MB
in f16) instead of 4, keeping the kernel compute-bound.

v2 schedule notes (from trace analysis of the v1 267 us baseline):
  - PE steady state was already at the 216 ns/MM issue-rate roofline; the
    ~45 us of slack was startup (12 us serialized warmup chain + W-ring
    starvation while the bias broadcast DMA burned ~150 GB/s) and tail.
  - bias now rides the scalar HWDGE ring in f16 (0.5 MB/slot, after xt),
    leaving the sync ring 100% for W from t=0; gpsimd SWDGE is unused.
  - xt is host-packed kt-major so each batch is one DMA with 8 KB
    contiguous per-partition rows (was 1 KB packets).
  - warmup is 20 short (N=256) matmuls round-robin over all 8 PSUM banks
    (pipelined, ~220 ns each warm / 430 cold) instead of 22 serialized
    same-bank matmuls; it just bridges HAM's ~3.4 us busy window until the
    first xt+W tiles land.
  - fp8 was considered and rejected: max-rel-err would be ~5e-2 > 2e-2 gate.

Per core (slot-major):
  for slot s in [A(2 batches), B(1), C(1)]:
    for half (2 x 2048 cols):
      stream W[s]-half as 8 k-tiles [128, 2048] f16 on the sync HWDGE ring
      for m over the slot's 128-sample tiles (8 for A, 4 for B/C):
        for kt(8): 4 matmuls (2 psum tiles [128,1024] x 2 banks), accum kt
        evict psum + bias (2 DVE adds) -> out tile, DMA to out (scalar ring)
"""

import numpy as np

import concourse.bacc as bacc
import concourse.mybir as mybir
import concourse.bass as bass
import concourse.tile as tile
from concourse.bass_utils import run_bass_kernel_spmd

N_CORES = 8
B, S, K, H = 32, 512, 1024, 4096
BPC = B // N_CORES          # batches per core
P = 128                     # partitions
KT = K // P                 # k tiles (8)
MT = S // P                 # sample tiles per batch (4)
NHALF = 2                   # n halves
NH = H // NHALF             # cols per half (2048)
SLOT_BATCHES = (2, 1, 1)    # batches per weight slot
NSLOT = len(SLOT_BATCHES)
N_WARM = 14                 # warmup matmuls (N=256, round robin over 8 banks)

_COMPILED = None


def _build():
    nc = bacc.Bacc("TRN2", target_bir_lowering=False, debug=False)
    f32 = mybir.dt.float32
    f16 = mybir.dt.float16

    # xt: per batch, partition p holds x[b, :, kt*128+p] for kt=0..7, i.e.
    # row layout [kt, m] (8 KB contiguous per partition row).
    xt_ap = nc.dram_tensor("xt", [BPC, P, KT * S], f16, kind="ExternalInput").ap()
    # w: [slot, half, kt, p, n] so each (slot, half, kt) tile is [128, 2048]
    # with 4 KB contiguous per-partition rows.
    w_ap = nc.dram_tensor(
        "w", [NSLOT, NHALF, KT, P, NH], f16, kind="ExternalInput"
    ).ap()
    bias_ap = nc.dram_tensor("bias", [NSLOT, H], f16, kind="ExternalInput").ap()
    out_ap = nc.dram_tensor("out", [BPC, S, H], f32, kind="ExternalOutput").ap()

    with tile.TileContext(nc) as tc:
        with (
            tc.tile_pool(name="xt_pool", bufs=4) as xt_pool,
            tc.tile_pool(name="w_pool", bufs=16) as w_pool,
            tc.tile_pool(name="bias_pool", bufs=2) as bias_pool,
            tc.tile_pool(name="out_pool", bufs=4) as out_pool,
            tc.tile_pool(name="ps_pool", bufs=4, space="PSUM") as ps_pool,
        ):
            # Allocate xt and bias tiles up front; DMA-issue order on the
            # scalar ring is staged so only what gates the first slot rides
            # the front of the ring: [w(slot0,h0,kt0), xt_b0, bias_A, xt_b1].
            # xt_b2/b3 and bias_B/C are emitted between early out-stores so
            # they can't starve the W (sync) ring during startup.
            xt_ts = [
                xt_pool.tile([P, KT * S], f16, name="xt_t", tag="xt")
                for _ in range(BPC)
            ]
            bias_ts = [
                bias_pool.tile([P, H], f16, name="bias_t") for _ in range(NSLOT)
            ]

            def dma_xt(b):
                nc.scalar.dma_start(xt_ts[b][:], xt_ap[b])

            def dma_bias(s):
                bias_src = bias_ap[s]
                nc.scalar.dma_start(
                    bias_ts[s][:],
                    bass.AP(
                        tensor=bias_src.tensor,
                        offset=bias_src.offset,
                        ap=[[0, P]] + list(bias_src.ap),
                    ),
                )

            # First W tile of slot 0 rides the scalar ring ahead of xt so the
            # PE's first k-group isn't waiting behind 1 MB of xt.
            w00_kt0 = w_pool.tile([P, NH], f16, tag="w", name="w_t")
            nc.scalar.dma_start(w00_kt0[:], w_ap[0, 0, 0])
            dma_xt(0)
            dma_bias(0)
            dma_xt(1)

            # Warm up the PE (HAM un-throttle) while the first DMAs land:
            # short matmuls on memset tiles, round-robin over all 8 PSUM banks
            # so they pipeline at issue rate instead of serializing on one
            # bank. Results read once so DCE keeps them.
            warm_x = xt_pool.tile([P, P], f16, name="warm_x", tag="warm")
            warm_w = w_pool.tile([P, 256], f16, tag="warmw", name="warm_w")
            nc.vector.memset(warm_x[:], 0.0)
            nc.vector.memset(warm_w[:], 0.0)
            warm_ps = [
                ps_pool.tile([P, 1024], f32, tag="ps", name="warm_ps")
                for _ in range(4)
            ]
            for i in range(N_WARM):
                t = warm_ps[(i // 2) % 4]
                col = 512 * (i % 2)
                nc.tensor.matmul(
                    t[:, col : col + 256], warm_x[:], warm_w[:],
                    start=True, stop=True, skip_group_check=True,
                )
            warm_out = out_pool.tile([P, 16], f32, name="warm_out", tag="warmo")
            for i in range(4):
                nc.vector.tensor_copy(warm_out[:, 4 * i : 4 * i + 4], warm_ps[i][:, 0:4])

            # After out-store #k on the scalar ring, emit these deferred DMAs.
            deferred = {0: lambda: dma_xt(2), 1: lambda: dma_xt(3),
                        2: lambda: dma_bias(1), 8: lambda: dma_bias(2)}
            store_idx = 0

            bi0 = 0  # first batch index of this slot
            for s in range(NSLOT):
                nb = SLOT_BATCHES[s]
                for half in range(NHALF):
                    w_tiles = []
                    for kt in range(KT):
                        if s == 0 and half == 0 and kt == 0:
                            w_tiles.append(w00_kt0)
                            continue
                        w_t = w_pool.tile([P, NH], f16, tag="w", name="w_t")
                        nc.sync.dma_start(w_t[:], w_ap[s, half, kt])
                        w_tiles.append(w_t)
                    for m in range(nb * MT):
                        b, mm = divmod(m, MT)
                        last_iter = (
                            s == NSLOT - 1 and half == NHALF - 1 and m == nb * MT - 1
                        )
                        ps = [
                            ps_pool.tile([P, 1024], f32, tag="ps", name="ps")
                            for _ in range(2)
                        ]
                        if last_iter:
                            # n-major so ps0 finishes early: evict + store it
                            # while ps1's matmuls still run (shorter tail).
                            mm_order = [
                                (kt, n4) for n4 in range(4) for kt in range(KT)
                            ]
                        else:
                            mm_order = [
                                (kt, n4) for kt in range(KT) for n4 in range(4)
                            ]
                        for kt, n4 in mm_order:
                            lhsT = xt_ts[bi0 + b][
                                :, kt * S + mm * P : kt * S + (mm + 1) * P
                            ]
                            nc.tensor.matmul(
                                ps[n4 // 2][:, (n4 % 2) * 512 : (n4 % 2) * 512 + 512],
                                lhsT,
                                w_tiles[kt][:, n4 * 512 : (n4 + 1) * 512],
                                start=(kt == 0),
                                stop=(kt == KT - 1),
                            )
                        out_t = out_pool.tile([P, NH], f32)
                        for h2 in range(2):
                            nc.vector.tensor_add(
                                out_t[:, h2 * 1024 : (h2 + 1) * 1024],
                                ps[h2][:],
                                bias_ts[s][
                                    :, half * NH + h2 * 1024 : half * NH + (h2 + 1) * 1024
                                ],
                            )
                            if last_iter:
                                nc.scalar.dma_start(
                                    out_ap[
                                        bi0 + b,
                                        mm * P : (mm + 1) * P,
                                        half * NH + h2 * 1024 : half * NH + (h2 + 1) * 1024,
                                    ],
                                    out_t[:, h2 * 1024 : (h2 + 1) * 1024],
                                )
                        if not last_iter:
                            nc.scalar.dma_start(
                                out_ap[
                                    bi0 + b,
                                    mm * P : (mm + 1) * P,
                                    half * NH : (half + 1) * NH,
                                ],
                                out_t[:],
                            )
                        cb = deferred.pop(store_idx, None)
                        if cb is not None:
                            cb()
                        store_idx += 1
                bi0 += nb
    nc.compile()
    return nc


def _get_compiled():
    global _COMPILED
    if _COMPILED is None:
        _COMPILED = _build()
    return _COMPILED


def _pack(cat_ids):
    """Assign batches to cores with slot capacities [2,1,1] per core.

    Returns per-core (idx, slot_cats): idx = 4 batch indices ordered
    [pair0, pair1, single_b, single_c]; slot_cats = categories for the 3 slots.
    Always feasible: #disjoint same-cat pairs = (32 - #odd-count cats)/2 >= 8.
    """
    cat_ids = np.asarray(cat_ids)
    by_cat = {}
    for i, c in enumerate(cat_ids.tolist()):
        by_cat.setdefault(c, []).append(i)
    pairs = []
    singles = []
    for c, idxs in sorted(by_cat.items()):
        n = len(idxs)
        for j in range(n // 2):
            pairs.append((c, idxs[2 * j], idxs[2 * j + 1]))
        if n % 2:
            singles.append((c, idxs[-1]))
    assert len(pairs) >= N_CORES, "impossible: <8 same-cat pairs among 32 batches"
    core_pairs = pairs[:N_CORES]
    # leftovers: extra pairs flatten into singles
    for c, i, j in pairs[N_CORES:]:
        singles.append((c, i))
        singles.append((c, j))
    assert len(singles) == 2 * N_CORES
    cores = []
    for ci in range(N_CORES):
        c, i, j = core_pairs[ci]
        (cb, ib), (cc, ic) = singles[2 * ci], singles[2 * ci + 1]
        cores.append(([i, j, ib, ic], [c, cb, cc]))
    return cores


def _host_pack_xt(xb):
    """x batches (n, 512, 1024) f32 -> (n, 128, KT*S) f16, kt-major rows.

    xt[b, p, kt*512 + m] = x[b, m, kt*128 + p]
    """
    n = xb.shape[0]
    xt = xb.astype(np.float16).transpose(0, 2, 1)          # (n, K, S)
    xt = xt.reshape(n, KT, P, S).transpose(0, 2, 1, 3)     # (n, P, KT, S)
    return np.ascontiguousarray(xt.reshape(n, P, KT * S))


def _host_pack_w(Wsel):
    """W slots (3, 1024, 4096) f32 -> (3, NHALF, KT, P, NH) f16.

    w[s, h, kt, p, j] = W[s, kt*128 + p, h*2048 + j]
    """
    w = Wsel.astype(np.float16).reshape(NSLOT, KT, P, NHALF, NH)
    return np.ascontiguousarray(w.transpose(0, 3, 1, 2, 4))


def run_sharded(x, cat_ids, W, b, trace=False, **spmd_kwargs):
    """Shard, run on 8 cores, unshard. Returns (out, BassKernelResults)."""
    x = np.ascontiguousarray(np.asarray(x), dtype=np.float32)
    cat_ids = np.asarray(cat_ids).astype(np.int64)
    W = np.ascontiguousarray(np.asarray(W), dtype=np.float32)
    b = np.ascontiguousarray(np.asarray(b), dtype=np.float32)

    nc = _get_compiled()
    cores = _pack(cat_ids)

    in_maps = []
    for idx, slot_cats in cores:
        in_maps.append(
            {
                "xt": _host_pack_xt(x[idx]),
                "w": _host_pack_w(W[slot_cats]),
                "bias": b[slot_cats].astype(np.float16),
            }
        )

    res = run_bass_kernel_spmd(
        nc, in_maps, list(range(N_CORES)), trace=trace, **spmd_kwargs
    )

    out = np.empty((B, S, H), dtype=np.float32)
    for c, (idx, _) in enumerate(cores):
        out[idx] = res.results[c]["out"]
    return out, res


def kernel(x, cat_ids, W, b):
    out, _ = run_sharded(x, cat_ids, W, b)
    return out


# revision 7
# speedup vs baseline: 1.0252x; 1.0252x over previous
"""Category-specific linear (MoE-style routed batched matmul) on 8 trn2 cores.

out[b, s, h] = sum_i x[b, s, i] * W[cat_ids[b], i, h] + bias[cat_ids[b], h]

Shapes (hardcoded): x (32, 512, 1024) f32, cat_ids (32,) int, W (16, 1024, 4096)
f32, b (16, 4096) f32 -> out (32, 512, 4096) f32.

Strategy: data-parallel over batch, 4 batches per core, with host-side routing
that always packs one same-category PAIR of batches plus two singles per core
(slot capacities [2, 1, 1] batches). With 32 batches over 16 categories there
are always >= (32 - 16)/2 = 8 disjoint same-category pairs, so this packing is
feasible for ANY cat_ids. Each core then loads only 3 weight matrices (24 MB
in f16) instead of 4, keeping the kernel compute-bound.

v2 schedule notes (from trace analysis of the v1 267 us baseline):
  - PE steady state was already at the 216 ns/MM issue-rate roofline; the
    ~45 us of slack was startup (12 us serialized warmup chain + W-ring
    starvation while the bias broadcast DMA burned ~150 GB/s) and tail.
  - bias now rides the scalar HWDGE ring in f16 (0.5 MB/slot, after xt),
    leaving the sync ring 100% for W from t=0; gpsimd SWDGE is unused.
  - xt is host-packed kt-major so each batch is one DMA with 8 KB
    contiguous per-partition rows (was 1 KB packets).
  - warmup is 20 short (N=256) matmuls round-robin over all 8 PSUM banks
    (pipelined, ~220 ns each warm / 430 cold) instead of 22 serialized
    same-bank matmuls; it just bridges HAM's ~3.4 us busy window until the
    first xt+W tiles land.
  - fp8 was considered and rejected: max-rel-err would be ~5e-2 > 2e-2 gate.

Per core (slot-major):
  for slot s in [A(2 batches), B(1), C(1)]:
    for half (2 x 2048 cols):
      stream W[s]-half as 8 k-tiles [128, 2048] f16 on the sync HWDGE ring
      for m over the slot's 128-sample tiles (8 for A, 4 for B/C):
        for kt(8): 4 matmuls (2 psum tiles [128,1024] x 2 banks), accum kt
        evict psum + bias (2 DVE adds) -> out tile, DMA to out (scalar ring)
"""

import numpy as np

import concourse.bacc as bacc
import concourse.mybir as mybir
import concourse.bass as bass
import concourse.tile as tile
from concourse.bass_utils import run_bass_kernel_spmd

N_CORES = 8
B, S, K, H = 32, 512, 1024, 4096
BPC = B // N_CORES          # batches per core
P = 128                     # partitions
KT = K // P                 # k tiles (8)
MT = S // P                 # sample tiles per batch (4)
NHALF = 2                   # n halves
NH = H // NHALF             # cols per half (2048)
SLOT_BATCHES = (2, 1, 1)    # batches per weight slot
NSLOT = len(SLOT_BATCHES)
N_WARM = 20                 # warmup matmuls (N=256, round robin over 8 banks)

_COMPILED = None


def _build():
    nc = bacc.Bacc("TRN2", target_bir_lowering=False, debug=False)
    f32 = mybir.dt.float32
    f16 = mybir.dt.float16

    # xt: per batch, partition p holds x[b, :, kt*128+p] for kt=0..7, i.e.
    # row layout [kt, m] (8 KB contiguous per partition row).
    xt_ap = nc.dram_tensor("xt", [BPC, P, KT * S], f16, kind="ExternalInput").ap()
    # w: [slot, half, kt, p, n] so each (slot, half, kt) tile is [128, 2048]
    # with 4 KB contiguous per-partition rows.
    w_ap = nc.dram_tensor(
        "w", [NSLOT, NHALF, KT, P, NH], f16, kind="ExternalInput"
    ).ap()
    bias_ap = nc.dram_tensor("bias", [NSLOT, H], f16, kind="ExternalInput").ap()
    out_ap = nc.dram_tensor("out", [BPC, S, H], f32, kind="ExternalOutput").ap()

    with tile.TileContext(nc) as tc:
        with (
            tc.tile_pool(name="xt_pool", bufs=4) as xt_pool,
            tc.tile_pool(name="w_pool", bufs=16) as w_pool,
            tc.tile_pool(name="bias_pool", bufs=2) as bias_pool,
            tc.tile_pool(name="out_pool", bufs=4) as out_pool,
            tc.tile_pool(name="ps_pool", bufs=4, space="PSUM") as ps_pool,
        ):
            # Allocate xt and bias tiles up front. xt_b0 gates the very first
            # matmul, and the sync ring wakes ~4 us before the scalar ring,
            # so xt_b0 rides the sync ring ahead of all W. Everything not
            # needed in the first ~30 us is deferred via tile_wait_until so
            # the scheduler can't hoist it into the startup window.
            xt_ts = [
                xt_pool.tile([P, KT * S], f16, name="xt_t", tag="xt")
                for _ in range(BPC)
            ]
            bias_ts = [
                bias_pool.tile([P, H], f16, name="bias_t") for _ in range(NSLOT)
            ]

            def dma_xt(b, eng):
                eng.dma_start(xt_ts[b][:], xt_ap[b])

            def dma_bias(s):
                bias_src = bias_ap[s]
                nc.scalar.dma_start(
                    bias_ts[s][:],
                    bass.AP(
                        tensor=bias_src.tensor,
                        offset=bias_src.offset,
                        ap=[[0, P]] + list(bias_src.ap),
                    ),
                )

            dma_xt(0, nc.sync)       # sync ring front, before all W
            dma_bias(0)              # scalar ring front
            dma_xt(1, nc.scalar)
            with tc.tile_wait_until(0.030):
                dma_xt(2, nc.scalar)
            with tc.tile_wait_until(0.040):
                dma_xt(3, nc.scalar)
            with tc.tile_wait_until(0.050):
                dma_bias(1)
            with tc.tile_wait_until(0.070):
                dma_bias(2)

            # Warm up the PE (HAM un-throttle) while the first DMAs land:
            # short matmuls on memset tiles, round-robin over all 8 PSUM banks
            # so they pipeline at issue rate instead of serializing on one
            # bank. Results read once so DCE keeps them.
            warm_x = xt_pool.tile([P, P], f16, name="warm_x", tag="warm")
            warm_w = w_pool.tile([P, 256], f16, tag="warmw", name="warm_w")
            nc.vector.memset(warm_x[:], 0.0)
            nc.vector.memset(warm_w[:], 0.0)
            warm_ps = [
                ps_pool.tile([P, 1024], f32, tag="ps", name="warm_ps")
                for _ in range(4)
            ]
            for i in range(N_WARM):
                t = warm_ps[(i // 2) % 4]
                col = 512 * (i % 2)
                nc.tensor.matmul(
                    t[:, col : col + 256], warm_x[:], warm_w[:],
                    start=True, stop=True, skip_group_check=True,
                )
            warm_out = out_pool.tile([P, 16], f32, name="warm_out", tag="warmo")
            for i in range(4):
                nc.vector.tensor_copy(warm_out[:, 4 * i : 4 * i + 4], warm_ps[i][:, 0:4])

            # Phases: slot A (2 batches) runs 4 quarter-width (1024-col)
            # phases so the DMA-gated first m-sweep needs only 2 MB of W;
            # single-batch slots B/C run 2 half-width (2048-col) phases.
            phases = []
            bi0 = 0
            for s in range(NSLOT):
                nb = SLOT_BATCHES[s]
                ncols = 1024 if s == 0 else NH
                for col0 in range(0, H, ncols):
                    phases.append((s, bi0, nb, col0, ncols))
                bi0 += nb

            for pi, (s, bi0, nb, col0, ncols) in enumerate(phases):
                half, off = divmod(col0, NH)
                w_tiles = []
                for kt in range(KT):
                    w_t = w_pool.tile([P, ncols], f16, tag="w", name="w_t")
                    nc.sync.dma_start(
                        w_t[:], w_ap[s, half, kt, :, off : off + ncols]
                    )
                    w_tiles.append(w_t)
                nps = ncols // 1024
                for m in range(nb * MT):
                    b, mm = divmod(m, MT)
                    last_iter = pi == len(phases) - 1 and m == nb * MT - 1
                    ps = [
                        ps_pool.tile([P, 1024], f32, tag="ps", name="ps")
                        for _ in range(nps)
                    ]
                    if last_iter:
                        # n-major so ps0 finishes early: evict + store it
                        # while ps1's matmuls still run (shorter tail).
                        mm_order = [
                            (kt, n4) for n4 in range(2 * nps) for kt in range(KT)
                        ]
                    else:
                        mm_order = [
                            (kt, n4) for kt in range(KT) for n4 in range(2 * nps)
                        ]
                    for kt, n4 in mm_order:
                        lhsT = xt_ts[bi0 + b][
                            :, kt * S + mm * P : kt * S + (mm + 1) * P
                        ]
                        nc.tensor.matmul(
                            ps[n4 // 2][:, (n4 % 2) * 512 : (n4 % 2) * 512 + 512],
                            lhsT,
                            w_tiles[kt][:, n4 * 512 : (n4 + 1) * 512],
                            start=(kt == 0),
                            stop=(kt == KT - 1),
                        )
                    out_t = out_pool.tile([P, ncols], f32, tag="o", name="out_t")
                    for h2 in range(nps):
                        nc.vector.tensor_add(
                            out_t[:, h2 * 1024 : (h2 + 1) * 1024],
                            ps[h2][:],
                            bias_ts[s][:, col0 + h2 * 1024 : col0 + (h2 + 1) * 1024],
                        )
                        if last_iter or h2 == nps - 1:
                            c0 = 0 if (nps == 1 or not last_iter) else h2 * 1024
                            c1 = (h2 + 1) * 1024 if last_iter else ncols
                            nc.scalar.dma_start(
                                out_ap[
                                    bi0 + b,
                                    mm * P : (mm + 1) * P,
                                    col0 + c0 : col0 + c1,
                                ],
                                out_t[:, c0:c1],
                            )
    nc.compile()
    return nc


def _get_compiled():
    global _COMPILED
    if _COMPILED is None:
        _COMPILED = _build()
    return _COMPILED


def _pack(cat_ids):
    """Assign batches to cores with slot capacities [2,1,1] per core.

    Returns per-core (idx, slot_cats): idx = 4 batch indices ordered
    [pair0, pair1, single_b, single_c]; slot_cats = categories for the 3 slots.
    Always feasible: #disjoint same-cat pairs = (32 - #odd-count cats)/2 >= 8.
    """
    cat_ids = np.asarray(cat_ids)
    by_cat = {}
    for i, c in enumerate(cat_ids.tolist()):
        by_cat.setdefault(c, []).append(i)
    pairs = []
    singles = []
    for c, idxs in sorted(by_cat.items()):
        n = len(idxs)
        for j in range(n // 2):
            pairs.append((c, idxs[2 * j], idxs[2 * j + 1]))
        if n % 2:
            singles.append((c, idxs[-1]))
    assert len(pairs) >= N_CORES, "impossible: <8 same-cat pairs among 32 batches"
    core_pairs = pairs[:N_CORES]
    # leftovers: extra pairs flatten into singles
    for c, i, j in pairs[N_CORES:]:
        singles.append((c, i))
        singles.append((c, j))
    assert len(singles) == 2 * N_CORES
    cores = []
    for ci in range(N_CORES):
        c, i, j = core_pairs[ci]
        (cb, ib), (cc, ic) = singles[2 * ci], singles[2 * ci + 1]
        cores.append(([i, j, ib, ic], [c, cb, cc]))
    return cores


def _host_pack_xt(xb):
    """x batches (n, 512, 1024) f32 -> (n, 128, KT*S) f16, kt-major rows.

    xt[b, p, kt*512 + m] = x[b, m, kt*128 + p]
    """
    n = xb.shape[0]
    xt = xb.astype(np.float16).transpose(0, 2, 1)          # (n, K, S)
    xt = xt.reshape(n, KT, P, S).transpose(0, 2, 1, 3)     # (n, P, KT, S)
    return np.ascontiguousarray(xt.reshape(n, P, KT * S))


def _host_pack_w(Wsel):
    """W slots (3, 1024, 4096) f32 -> (3, NHALF, KT, P, NH) f16.

    w[s, h, kt, p, j] = W[s, kt*128 + p, h*2048 + j]
    """
    w = Wsel.astype(np.float16).reshape(NSLOT, KT, P, NHALF, NH)
    return np.ascontiguousarray(w.transpose(0, 3, 1, 2, 4))


def run_sharded(x, cat_ids, W, b, trace=False, **spmd_kwargs):
    """Shard, run on 8 cores, unshard. Returns (out, BassKernelResults)."""
    x = np.ascontiguousarray(np.asarray(x), dtype=np.float32)
    cat_ids = np.asarray(cat_ids).astype(np.int64)
    W = np.ascontiguousarray(np.asarray(W), dtype=np.float32)
    b = np.ascontiguousarray(np.asarray(b), dtype=np.float32)

    nc = _get_compiled()
    cores = _pack(cat_ids)

    in_maps = []
    for idx, slot_cats in cores:
        in_maps.append(
            {
                "xt": _host_pack_xt(x[idx]),
                "w": _host_pack_w(W[slot_cats]),
                "bias": b[slot_cats].astype(np.float16),
            }
        )

    res = run_bass_kernel_spmd(
        nc, in_maps, list(range(N_CORES)), trace=trace, **spmd_kwargs
    )

    out = np.empty((B, S, H), dtype=np.float32)
    for c, (idx, _) in enumerate(cores):
        out[idx] = res.results[c]["out"]
    return out, res


def kernel(x, cat_ids, W, b):
    out, _ = run_sharded(x, cat_ids, W, b)
    return out


# revision 10
# speedup vs baseline: 1.0264x; 1.0012x over previous
"""Category-specific linear (MoE-style routed batched matmul) on 8 trn2 cores.

out[b, s, h] = sum_i x[b, s, i] * W[cat_ids[b], i, h] + bias[cat_ids[b], h]

Shapes (hardcoded): x (32, 512, 1024) f32, cat_ids (32,) int, W (16, 1024, 4096)
f32, b (16, 4096) f32 -> out (32, 512, 4096) f32.

Strategy: data-parallel over batch, 4 batches per core, with host-side routing
that always packs one same-category PAIR of batches plus two singles per core
(slot capacities [2, 1, 1] batches). With 32 batches over 16 categories there
are always >= (32 - 16)/2 = 8 disjoint same-category pairs, so this packing is
feasible for ANY cat_ids. Each core then loads only 3 weight matrices (24 MB
in f16) instead of 4, keeping the kernel compute-bound.

v2 schedule notes (from trace analysis of the v1 267 us baseline):
  - PE steady state was already at the 216 ns/MM issue-rate roofline; the
    ~45 us of slack was startup (12 us serialized warmup chain + W-ring
    starvation while the bias broadcast DMA burned ~150 GB/s) and tail.
  - bias now rides the scalar HWDGE ring in f16 (0.5 MB/slot, after xt),
    leaving the sync ring 100% for W from t=0; gpsimd SWDGE is unused.
  - xt is host-packed kt-major so each batch is one DMA with 8 KB
    contiguous per-partition rows (was 1 KB packets).
  - warmup is 20 short (N=256) matmuls round-robin over all 8 PSUM banks
    (pipelined, ~220 ns each warm / 430 cold) instead of 22 serialized
    same-bank matmuls; it just bridges HAM's ~3.4 us busy window until the
    first xt+W tiles land.
  - fp8 was considered and rejected: max-rel-err would be ~5e-2 > 2e-2 gate.

Per core (slot-major):
  for slot s in [A(2 batches), B(1), C(1)]:
    for half (2 x 2048 cols):
      stream W[s]-half as 8 k-tiles [128, 2048] f16 on the sync HWDGE ring
      for m over the slot's 128-sample tiles (8 for A, 4 for B/C):
        for kt(8): 4 matmuls (2 psum tiles [128,1024] x 2 banks), accum kt
        evict psum + bias (2 DVE adds) -> out tile, DMA to out (scalar ring)
"""

import numpy as np

import concourse.bacc as bacc
import concourse.mybir as mybir
import concourse.bass as bass
import concourse.tile as tile
from concourse.bass_utils import run_bass_kernel_spmd

N_CORES = 8
B, S, K, H = 32, 512, 1024, 4096
BPC = B // N_CORES          # batches per core
P = 128                     # partitions
KT = K // P                 # k tiles (8)
MT = S // P                 # sample tiles per batch (4)
NHALF = 2                   # n halves
NH = H // NHALF             # cols per half (2048)
SLOT_BATCHES = (2, 1, 1)    # batches per weight slot
NSLOT = len(SLOT_BATCHES)
N_WARM = 20                 # warmup matmuls (N=256, round robin over 8 banks)

_COMPILED = None


def _build():
    nc = bacc.Bacc("TRN2", target_bir_lowering=False, debug=False)
    f32 = mybir.dt.float32
    f16 = mybir.dt.float16

    # xt: per batch, partition p holds x[b, :, kt*128+p] for kt=0..7, i.e.
    # row layout [kt, m] (8 KB contiguous per partition row).
    xt_ap = nc.dram_tensor("xt", [BPC, P, KT * S], f16, kind="ExternalInput").ap()
    # w: [slot, half, kt, p, n] so each (slot, half, kt) tile is [128, 2048]
    # with 4 KB contiguous per-partition rows.
    w_ap = nc.dram_tensor(
        "w", [NSLOT, NHALF, KT, P, NH], f16, kind="ExternalInput"
    ).ap()
    bias_ap = nc.dram_tensor("bias", [NSLOT, H], f16, kind="ExternalInput").ap()
    out_ap = nc.dram_tensor("out", [BPC, S, H], f32, kind="ExternalOutput").ap()

    with tile.TileContext(nc) as tc:
        with (
            tc.tile_pool(name="xt_pool", bufs=4) as xt_pool,
            tc.tile_pool(name="w_pool", bufs=16) as w_pool,
            tc.tile_pool(name="bias_pool", bufs=2) as bias_pool,
            tc.tile_pool(name="out_pool", bufs=4) as out_pool,
            tc.tile_pool(name="ps_pool", bufs=4, space="PSUM") as ps_pool,
        ):
            # Allocate xt and bias tiles up front. xt_b0 gates the very first
            # matmul, and the sync ring wakes ~4 us before the scalar ring,
            # so xt_b0 rides the sync ring ahead of all W. Everything not
            # needed in the first ~30 us is deferred via tile_wait_until so
            # the scheduler can't hoist it into the startup window.
            xt_ts = [
                xt_pool.tile([P, KT * S], f16, name="xt_t", tag="xt")
                for _ in range(BPC)
            ]
            bias_ts = [
                bias_pool.tile([P, H], f16, name="bias_t") for _ in range(NSLOT)
            ]

            def dma_xt(b, eng):
                eng.dma_start(xt_ts[b][:], xt_ap[b])

            def dma_bias(s):
                bias_src = bias_ap[s]
                nc.scalar.dma_start(
                    bias_ts[s][:],
                    bass.AP(
                        tensor=bias_src.tensor,
                        offset=bias_src.offset,
                        ap=[[0, P]] + list(bias_src.ap),
                    ),
                )

            dma_xt(0, nc.sync)       # sync ring front, before all W
            # Phase-0 W tiles (slot A, cols 0:1024) split across BOTH rings:
            # even kt on the scalar ring (its front), odd kt on the sync ring
            # behind xt_b0 - so the whole first m-sweep's W lands by ~16 us.
            w_tiles_p0 = []
            for kt in range(KT):
                w_t = w_pool.tile([P, 1024], f16, tag="w", name="w_t")
                eng = nc.scalar if kt % 2 == 0 else nc.sync
                eng.dma_start(w_t[:], w_ap[0, 0, kt, :, 0:1024])
                w_tiles_p0.append(w_t)
            dma_bias(0)              # scalar ring, behind even-kt W
            dma_xt(1, nc.scalar)
            with tc.tile_wait_until(0.030):
                dma_xt(2, nc.scalar)
            with tc.tile_wait_until(0.040):
                dma_xt(3, nc.scalar)
            with tc.tile_wait_until(0.050):
                dma_bias(1)
            with tc.tile_wait_until(0.070):
                dma_bias(2)

            # Warm up the PE (HAM un-throttle) while the first DMAs land:
            # short matmuls on memset tiles, round-robin over all 8 PSUM banks
            # so they pipeline at issue rate instead of serializing on one
            # bank. Results read once so DCE keeps them.
            warm_x = xt_pool.tile([P, P], f16, name="warm_x", tag="warm")
            warm_w = w_pool.tile([P, 256], f16, tag="warmw", name="warm_w")
            nc.vector.memset(warm_x[:], 0.0)
            nc.vector.memset(warm_w[:], 0.0)
            warm_ps = [
                ps_pool.tile([P, 1024], f32, tag="ps", name="warm_ps")
                for _ in range(4)
            ]
            for i in range(N_WARM):
                t = warm_ps[(i // 2) % 4]
                col = 512 * (i % 2)
                nc.tensor.matmul(
                    t[:, col : col + 256], warm_x[:], warm_w[:],
                    start=True, stop=True, skip_group_check=True,
                )
            warm_out = out_pool.tile([P, 16], f32, name="warm_out", tag="warmo")
            for i in range(4):
                nc.vector.tensor_copy(warm_out[:, 4 * i : 4 * i + 4], warm_ps[i][:, 0:4])

            # Phases: slot A (2 batches) runs 4 quarter-width (1024-col)
            # phases so the DMA-gated first m-sweep needs only 2 MB of W;
            # single-batch slots B/C run 2 half-width (2048-col) phases.
            phases = []
            bi0 = 0
            for s in range(NSLOT):
                nb = SLOT_BATCHES[s]
                ncols = 1024 if s == 0 else NH
                for col0 in range(0, H, ncols):
                    phases.append((s, bi0, nb, col0, ncols))
                bi0 += nb

            for pi, (s, bi0, nb, col0, ncols) in enumerate(phases):
                half, off = divmod(col0, NH)
                if pi == 0:
                    w_tiles = w_tiles_p0
                else:
                    w_tiles = []
                    for kt in range(KT):
                        w_t = w_pool.tile([P, ncols], f16, tag="w", name="w_t")
                        nc.sync.dma_start(
                            w_t[:], w_ap[s, half, kt, :, off : off + ncols]
                        )
                        w_tiles.append(w_t)
                nps = ncols // 1024
                for m in range(nb * MT):
                    b, mm = divmod(m, MT)
                    last_iter = pi == len(phases) - 1 and m == nb * MT - 1
                    ps = [
                        ps_pool.tile([P, 1024], f32, tag="ps", name="ps")
                        for _ in range(nps)
                    ]
                    if last_iter:
                        # n-major so ps0 finishes early: evict + store it
                        # while ps1's matmuls still run (shorter tail).
                        mm_order = [
                            (kt, n4) for n4 in range(2 * nps) for kt in range(KT)
                        ]
                    else:
                        mm_order = [
                            (kt, n4) for kt in range(KT) for n4 in range(2 * nps)
                        ]
                    for kt, n4 in mm_order:
                        lhsT = xt_ts[bi0 + b][
                            :, kt * S + mm * P : kt * S + (mm + 1) * P
                        ]
                        nc.tensor.matmul(
                            ps[n4 // 2][:, (n4 % 2) * 512 : (n4 % 2) * 512 + 512],
                            lhsT,
                            w_tiles[kt][:, n4 * 512 : (n4 + 1) * 512],
                            start=(kt == 0),
                            stop=(kt == KT - 1),
                        )
                    out_t = out_pool.tile([P, ncols], f32, tag="o", name="out_t")
                    if last_iter:
                        # Fine-grained (512-col) evict+store pipeline: each
                        # chunk's add and store start as soon as its bank's
                        # accumulation (n-major order) completes.
                        for q in range(2 * nps):
                            nc.vector.tensor_add(
                                out_t[:, q * 512 : (q + 1) * 512],
                                ps[q // 2][:, (q % 2) * 512 : (q % 2) * 512 + 512],
                                bias_ts[s][:, col0 + q * 512 : col0 + (q + 1) * 512],
                            )
                            nc.scalar.dma_start(
                                out_ap[
                                    bi0 + b,
                                    mm * P : (mm + 1) * P,
                                    col0 + q * 512 : col0 + (q + 1) * 512,
                                ],
                                out_t[:, q * 512 : (q + 1) * 512],
                            )
                    else:
                        for h2 in range(nps):
                            nc.vector.tensor_add(
                                out_t[:, h2 * 1024 : (h2 + 1) * 1024],
                                ps[h2][:],
                                bias_ts[s][:, col0 + h2 * 1024 : col0 + (h2 + 1) * 1024],
                            )
                        nc.scalar.dma_start(
                            out_ap[
                                bi0 + b,
                                mm * P : (mm + 1) * P,
                                col0 : col0 + ncols,
                            ],
                            out_t[:],
                        )
    nc.compile()
    return nc


def _get_compiled():
    global _COMPILED
    if _COMPILED is None:
        _COMPILED = _build()
    return _COMPILED


def _pack(cat_ids):
    """Assign batches to cores with slot capacities [2,1,1] per core.

    Returns per-core (idx, slot_cats): idx = 4 batch indices ordered
    [pair0, pair1, single_b, single_c]; slot_cats = categories for the 3 slots.
    Always feasible: #disjoint same-cat pairs = (32 - #odd-count cats)/2 >= 8.
    """
    cat_ids = np.asarray(cat_ids)
    by_cat = {}
    for i, c in enumerate(cat_ids.tolist()):
        by_cat.setdefault(c, []).append(i)
    pairs = []
    singles = []
    for c, idxs in sorted(by_cat.items()):
        n = len(idxs)
        for j in range(n // 2):
            pairs.append((c, idxs[2 * j], idxs[2 * j + 1]))
        if n % 2:
            singles.append((c, idxs[-1]))
    assert len(pairs) >= N_CORES, "impossible: <8 same-cat pairs among 32 batches"
    core_pairs = pairs[:N_CORES]
    # leftovers: extra pairs flatten into singles
    for c, i, j in pairs[N_CORES:]:
        singles.append((c, i))
        singles.append((c, j))
    assert len(singles) == 2 * N_CORES
    cores = []
    for ci in range(N_CORES):
        c, i, j = core_pairs[ci]
        (cb, ib), (cc, ic) = singles[2 * ci], singles[2 * ci + 1]
        cores.append(([i, j, ib, ic], [c, cb, cc]))
    return cores


def _host_pack_xt(xb):
    """x batches (n, 512, 1024) f32 -> (n, 128, KT*S) f16, kt-major rows.

    xt[b, p, kt*512 + m] = x[b, m, kt*128 + p]
    """
    n = xb.shape[0]
    xt = xb.astype(np.float16).transpose(0, 2, 1)          # (n, K, S)
    xt = xt.reshape(n, KT, P, S).transpose(0, 2, 1, 3)     # (n, P, KT, S)
    return np.ascontiguousarray(xt.reshape(n, P, KT * S))


def _host_pack_w(Wsel):
    """W slots (3, 1024, 4096) f32 -> (3, NHALF, KT, P, NH) f16.

    w[s, h, kt, p, j] = W[s, kt*128 + p, h*2048 + j]
    """
    w = Wsel.astype(np.float16).reshape(NSLOT, KT, P, NHALF, NH)
    return np.ascontiguousarray(w.transpose(0, 3, 1, 2, 4))


def run_sharded(x, cat_ids, W, b, trace=False, **spmd_kwargs):
    """Shard, run on 8 cores, unshard. Returns (out, BassKernelResults)."""
    x = np.ascontiguousarray(np.asarray(x), dtype=np.float32)
    cat_ids = np.asarray(cat_ids).astype(np.int64)
    W = np.ascontiguousarray(np.asarray(W), dtype=np.float32)
    b = np.ascontiguousarray(np.asarray(b), dtype=np.float32)

    nc = _get_compiled()
    cores = _pack(cat_ids)

    in_maps = []
    for idx, slot_cats in cores:
        in_maps.append(
            {
                "xt": _host_pack_xt(x[idx]),
                "w": _host_pack_w(W[slot_cats]),
                "bias": b[slot_cats].astype(np.float16),
            }
        )

    res = run_bass_kernel_spmd(
        nc, in_maps, list(range(N_CORES)), trace=trace, **spmd_kwargs
    )

    out = np.empty((B, S, H), dtype=np.float32)
    for c, (idx, _) in enumerate(cores):
        out[idx] = res.results[c]["out"]
    return out, res


def kernel(x, cat_ids, W, b):
    out, _ = run_sharded(x, cat_ids, W, b)
    return out


# revision 11
# speedup vs baseline: 1.0276x; 1.0012x over previous
"""Category-specific linear (MoE-style routed batched matmul) on 8 trn2 cores.

out[b, s, h] = sum_i x[b, s, i] * W[cat_ids[b], i, h] + bias[cat_ids[b], h]

Shapes (hardcoded): x (32, 512, 1024) f32, cat_ids (32,) int, W (16, 1024, 4096)
f32, b (16, 4096) f32 -> out (32, 512, 4096) f32.

Strategy: data-parallel over batch, 4 batches per core, with host-side routing
that always packs one same-category PAIR of batches plus two singles per core
(slot capacities [2, 1, 1] batches). With 32 batches over 16 categories there
are always >= (32 - 16)/2 = 8 disjoint same-category pairs, so this packing is
feasible for ANY cat_ids. Each core then loads only 3 weight matrices (24 MB
in f16) instead of 4, keeping the kernel compute-bound.

v2 schedule notes (from trace analysis of the v1 267 us baseline):
  - PE steady state was already at the 216 ns/MM issue-rate roofline; the
    ~45 us of slack was startup (12 us serialized warmup chain + W-ring
    starvation while the bias broadcast DMA burned ~150 GB/s) and tail.
  - bias now rides the scalar HWDGE ring in f16 (0.5 MB/slot, after xt),
    leaving the sync ring 100% for W from t=0; gpsimd SWDGE is unused.
  - xt is host-packed kt-major so each batch is one DMA with 8 KB
    contiguous per-partition rows (was 1 KB packets).
  - warmup is 20 short (N=256) matmuls round-robin over all 8 PSUM banks
    (pipelined, ~220 ns each warm / 430 cold) instead of 22 serialized
    same-bank matmuls; it just bridges HAM's ~3.4 us busy window until the
    first xt+W tiles land.
  - fp8 was considered and rejected: max-rel-err would be ~5e-2 > 2e-2 gate.

Per core (slot-major):
  for slot s in [A(2 batches), B(1), C(1)]:
    for half (2 x 2048 cols):
      stream W[s]-half as 8 k-tiles [128, 2048] f16 on the sync HWDGE ring
      for m over the slot's 128-sample tiles (8 for A, 4 for B/C):
        for kt(8): 4 matmuls (2 psum tiles [128,1024] x 2 banks), accum kt
        evict psum + bias (2 DVE adds) -> out tile, DMA to out (scalar ring)
"""

import numpy as np

import concourse.bacc as bacc
import concourse.mybir as mybir
import concourse.bass as bass
import concourse.tile as tile
from concourse.bass_utils import run_bass_kernel_spmd

N_CORES = 8
B, S, K, H = 32, 512, 1024, 4096
BPC = B // N_CORES          # batches per core
P = 128                     # partitions
KT = K // P                 # k tiles (8)
MT = S // P                 # sample tiles per batch (4)
NHALF = 2                   # n halves
NH = H // NHALF             # cols per half (2048)
SLOT_BATCHES = (2, 1, 1)    # batches per weight slot
NSLOT = len(SLOT_BATCHES)
N_WARM = 20                 # warmup matmuls (N=256, round robin over 8 banks)

_COMPILED = None


def _build():
    nc = bacc.Bacc("TRN2", target_bir_lowering=False, debug=False)
    f32 = mybir.dt.float32
    f16 = mybir.dt.float16

    # xt: per batch, partition p holds x[b, :, kt*128+p] for kt=0..7, i.e.
    # row layout [kt, m] (8 KB contiguous per partition row).
    xt_ap = nc.dram_tensor("xt", [BPC, P, KT * S], f16, kind="ExternalInput").ap()
    # w: [slot, half, kt, p, n] so each (slot, half, kt) tile is [128, 2048]
    # with 4 KB contiguous per-partition rows.
    w_ap = nc.dram_tensor(
        "w", [NSLOT, NHALF, KT, P, NH], f16, kind="ExternalInput"
    ).ap()
    bias_ap = nc.dram_tensor("bias", [NSLOT, H], f16, kind="ExternalInput").ap()
    out_ap = nc.dram_tensor("out", [BPC, S, H], f32, kind="ExternalOutput").ap()

    with tile.TileContext(nc) as tc:
        with (
            tc.tile_pool(name="xt_pool", bufs=4) as xt_pool,
            tc.tile_pool(name="w_pool", bufs=16) as w_pool,
            tc.tile_pool(name="bias_pool", bufs=2) as bias_pool,
            tc.tile_pool(name="out_pool", bufs=4) as out_pool,
            tc.tile_pool(name="ps_pool", bufs=4, space="PSUM") as ps_pool,
        ):
            # Allocate xt and bias tiles up front. xt_b0 gates the very first
            # matmul, and the sync ring wakes ~4 us before the scalar ring,
            # so xt_b0 rides the sync ring ahead of all W. Everything not
            # needed in the first ~30 us is deferred via tile_wait_until so
            # the scheduler can't hoist it into the startup window.
            xt_ts = [
                xt_pool.tile([P, KT * S], f16, name="xt_t", tag="xt")
                for _ in range(BPC)
            ]
            bias_ts = [
                bias_pool.tile([P, H], f16, name="bias_t") for _ in range(NSLOT)
            ]

            def dma_xt(b, eng):
                eng.dma_start(xt_ts[b][:], xt_ap[b])

            def dma_bias(s):
                bias_src = bias_ap[s]
                nc.scalar.dma_start(
                    bias_ts[s][:],
                    bass.AP(
                        tensor=bias_src.tensor,
                        offset=bias_src.offset,
                        ap=[[0, P]] + list(bias_src.ap),
                    ),
                )

            # xt_b0 gates the first matmuls: split it in two 0.5 MB chunks
            # (kt 0-3, kt 4-7) so kt0's matmuls only wait for chunk 0.
            hx = KT * S // 2
            nc.sync.dma_start(xt_ts[0][:, 0:hx], xt_ap[0][:, 0:hx])
            nc.sync.dma_start(xt_ts[0][:, hx:], xt_ap[0][:, hx:])
            # Phase-0 W tiles (slot A, cols 0:1024) split across BOTH rings:
            # even kt on the scalar ring (its front), odd kt on the sync ring
            # behind xt_b0 - so the whole first m-sweep's W lands by ~16 us.
            w_tiles_p0 = []
            for kt in range(KT):
                w_t = w_pool.tile([P, 1024], f16, tag="w", name="w_t")
                eng = nc.scalar if kt % 2 == 0 else nc.sync
                eng.dma_start(w_t[:], w_ap[0, 0, kt, :, 0:1024])
                w_tiles_p0.append(w_t)
            dma_bias(0)              # scalar ring, behind even-kt W
            dma_xt(1, nc.scalar)
            with tc.tile_wait_until(0.030):
                dma_xt(2, nc.scalar)
            with tc.tile_wait_until(0.040):
                dma_xt(3, nc.scalar)
            with tc.tile_wait_until(0.050):
                dma_bias(1)
            with tc.tile_wait_until(0.070):
                dma_bias(2)

            # Warm up the PE (HAM un-throttle) while the first DMAs land:
            # short matmuls on memset tiles, round-robin over all 8 PSUM banks
            # so they pipeline at issue rate instead of serializing on one
            # bank. Results read once so DCE keeps them.
            warm_x = xt_pool.tile([P, P], f16, name="warm_x", tag="warm")
            warm_w = w_pool.tile([P, 256], f16, tag="warmw", name="warm_w")
            nc.vector.memset(warm_x[:], 0.0)
            nc.vector.memset(warm_w[:], 0.0)
            warm_ps = [
                ps_pool.tile([P, 1024], f32, tag="ps", name="warm_ps")
                for _ in range(4)
            ]
            for i in range(N_WARM):
                t = warm_ps[(i // 2) % 4]
                col = 512 * (i % 2)
                nc.tensor.matmul(
                    t[:, col : col + 256], warm_x[:], warm_w[:],
                    start=True, stop=True, skip_group_check=True,
                )
            warm_out = out_pool.tile([P, 16], f32, name="warm_out", tag="warmo")
            for i in range(4):
                nc.vector.tensor_copy(warm_out[:, 4 * i : 4 * i + 4], warm_ps[i][:, 0:4])

            # Phases: slot A (2 batches) runs 4 quarter-width (1024-col)
            # phases so the DMA-gated first m-sweep needs only 2 MB of W;
            # single-batch slots B/C run 2 half-width (2048-col) phases.
            phases = []
            bi0 = 0
            for s in range(NSLOT):
                nb = SLOT_BATCHES[s]
                ncols = 1024 if s == 0 else NH
                for col0 in range(0, H, ncols):
                    phases.append((s, bi0, nb, col0, ncols))
                bi0 += nb

            for pi, (s, bi0, nb, col0, ncols) in enumerate(phases):
                half, off = divmod(col0, NH)
                if pi == 0:
                    w_tiles = w_tiles_p0
                else:
                    w_tiles = []
                    for kt in range(KT):
                        w_t = w_pool.tile([P, ncols], f16, tag="w", name="w_t")
                        nc.sync.dma_start(
                            w_t[:], w_ap[s, half, kt, :, off : off + ncols]
                        )
                        w_tiles.append(w_t)
                nps = ncols // 1024
                for m in range(nb * MT):
                    b, mm = divmod(m, MT)
                    last_iter = pi == len(phases) - 1 and m == nb * MT - 1
                    ps = [
                        ps_pool.tile([P, 1024], f32, tag="ps", name="ps")
                        for _ in range(nps)
                    ]
                    if last_iter:
                        # n-major so ps0 finishes early: evict + store it
                        # while ps1's matmuls still run (shorter tail).
                        mm_order = [
                            (kt, n4) for n4 in range(2 * nps) for kt in range(KT)
                        ]
                    else:
                        mm_order = [
                            (kt, n4) for kt in range(KT) for n4 in range(2 * nps)
                        ]
                    for kt, n4 in mm_order:
                        lhsT = xt_ts[bi0 + b][
                            :, kt * S + mm * P : kt * S + (mm + 1) * P
                        ]
                        nc.tensor.matmul(
                            ps[n4 // 2][:, (n4 % 2) * 512 : (n4 % 2) * 512 + 512],
                            lhsT,
                            w_tiles[kt][:, n4 * 512 : (n4 + 1) * 512],
                            start=(kt == 0),
                            stop=(kt == KT - 1),
                        )
                    out_t = out_pool.tile([P, ncols], f32, tag="o", name="out_t")
                    if last_iter:
                        # Fine-grained (512-col) evict+store pipeline: each
                        # chunk's add and store start as soon as its bank's
                        # accumulation (n-major order) completes.
                        for q in range(2 * nps):
                            nc.vector.tensor_add(
                                out_t[:, q * 512 : (q + 1) * 512],
                                ps[q // 2][:, (q % 2) * 512 : (q % 2) * 512 + 512],
                                bias_ts[s][:, col0 + q * 512 : col0 + (q + 1) * 512],
                            )
                            nc.scalar.dma_start(
                                out_ap[
                                    bi0 + b,
                                    mm * P : (mm + 1) * P,
                                    col0 + q * 512 : col0 + (q + 1) * 512,
                                ],
                                out_t[:, q * 512 : (q + 1) * 512],
                            )
                    else:
                        for h2 in range(nps):
                            nc.vector.tensor_add(
                                out_t[:, h2 * 1024 : (h2 + 1) * 1024],
                                ps[h2][:],
                                bias_ts[s][:, col0 + h2 * 1024 : col0 + (h2 + 1) * 1024],
                            )
                        nc.scalar.dma_start(
                            out_ap[
                                bi0 + b,
                                mm * P : (mm + 1) * P,
                                col0 : col0 + ncols,
                            ],
                            out_t[:],
                        )
    nc.compile()
    return nc


def _get_compiled():
    global _COMPILED
    if _COMPILED is None:
        _COMPILED = _build()
    return _COMPILED


def _pack(cat_ids):
    """Assign batches to cores with slot capacities [2,1,1] per core.

    Returns per-core (idx, slot_cats): idx = 4 batch indices ordered
    [pair0, pair1, single_b, single_c]; slot_cats = categories for the 3 slots.
    Always feasible: #disjoint same-cat pairs = (32 - #odd-count cats)/2 >= 8.
    """
    cat_ids = np.asarray(cat_ids)
    by_cat = {}
    for i, c in enumerate(cat_ids.tolist()):
        by_cat.setdefault(c, []).append(i)
    pairs = []
    singles = []
    for c, idxs in sorted(by_cat.items()):
        n = len(idxs)
        for j in range(n // 2):
            pairs.append((c, idxs[2 * j], idxs[2 * j + 1]))
        if n % 2:
            singles.append((c, idxs[-1]))
    assert len(pairs) >= N_CORES, "impossible: <8 same-cat pairs among 32 batches"
    core_pairs = pairs[:N_CORES]
    # leftovers: extra pairs flatten into singles
    for c, i, j in pairs[N_CORES:]:
        singles.append((c, i))
        singles.append((c, j))
    assert len(singles) == 2 * N_CORES
    cores = []
    for ci in range(N_CORES):
        c, i, j = core_pairs[ci]
        (cb, ib), (cc, ic) = singles[2 * ci], singles[2 * ci + 1]
        cores.append(([i, j, ib, ic], [c, cb, cc]))
    return cores


def _host_pack_xt(xb):
    """x batches (n, 512, 1024) f32 -> (n, 128, KT*S) f16, kt-major rows.

    xt[b, p, kt*512 + m] = x[b, m, kt*128 + p]
    """
    n = xb.shape[0]
    xt = xb.astype(np.float16).transpose(0, 2, 1)          # (n, K, S)
    xt = xt.reshape(n, KT, P, S).transpose(0, 2, 1, 3)     # (n, P, KT, S)
    return np.ascontiguousarray(xt.reshape(n, P, KT * S))


def _host_pack_w(Wsel):
    """W slots (3, 1024, 4096) f32 -> (3, NHALF, KT, P, NH) f16.

    w[s, h, kt, p, j] = W[s, kt*128 + p, h*2048 + j]
    """
    w = Wsel.astype(np.float16).reshape(NSLOT, KT, P, NHALF, NH)
    return np.ascontiguousarray(w.transpose(0, 3, 1, 2, 4))


def run_sharded(x, cat_ids, W, b, trace=False, **spmd_kwargs):
    """Shard, run on 8 cores, unshard. Returns (out, BassKernelResults)."""
    x = np.ascontiguousarray(np.asarray(x), dtype=np.float32)
    cat_ids = np.asarray(cat_ids).astype(np.int64)
    W = np.ascontiguousarray(np.asarray(W), dtype=np.float32)
    b = np.ascontiguousarray(np.asarray(b), dtype=np.float32)

    nc = _get_compiled()
    cores = _pack(cat_ids)

    in_maps = []
    for idx, slot_cats in cores:
        in_maps.append(
            {
                "xt": _host_pack_xt(x[idx]),
                "w": _host_pack_w(W[slot_cats]),
                "bias": b[slot_cats].astype(np.float16),
            }
        )

    res = run_bass_kernel_spmd(
        nc, in_maps, list(range(N_CORES)), trace=trace, **spmd_kwargs
    )

    out = np.empty((B, S, H), dtype=np.float32)
    for c, (idx, _) in enumerate(cores):
        out[idx] = res.results[c]["out"]
    return out, res


def kernel(x, cat_ids, W, b):
    out, _ = run_sharded(x, cat_ids, W, b)
    return out


# revision 12
# speedup vs baseline: 1.0306x; 1.0029x over previous
"""Category-specific linear (MoE-style routed batched matmul) on 8 trn2 cores.

out[b, s, h] = sum_i x[b, s, i] * W[cat_ids[b], i, h] + bias[cat_ids[b], h]

Shapes (hardcoded): x (32, 512, 1024) f32, cat_ids (32,) int, W (16, 1024, 4096)
f32, b (16, 4096) f32 -> out (32, 512, 4096) f32.

Strategy: data-parallel over batch, 4 batches per core, with host-side routing
that always packs one same-category PAIR of batches plus two singles per core
(slot capacities [2, 1, 1] batches). With 32 batches over 16 categories there
are always >= (32 - 16)/2 = 8 disjoint same-category pairs, so this packing is
feasible for ANY cat_ids. Each core then loads only 3 weight matrices (24 MB
in f16) instead of 4, keeping the kernel compute-bound.

v2 schedule notes (from trace analysis of the v1 267 us baseline):
  - PE steady state was already at the 216 ns/MM issue-rate roofline; the
    ~45 us of slack was startup (12 us serialized warmup chain + W-ring
    starvation while the bias broadcast DMA burned ~150 GB/s) and tail.
  - bias now rides the scalar HWDGE ring in f16 (0.5 MB/slot, after xt),
    leaving the sync ring 100% for W from t=0; gpsimd SWDGE is unused.
  - xt is host-packed kt-major so each batch is one DMA with 8 KB
    contiguous per-partition rows (was 1 KB packets).
  - warmup is 20 short (N=256) matmuls round-robin over all 8 PSUM banks
    (pipelined, ~220 ns each warm / 430 cold) instead of 22 serialized
    same-bank matmuls; it just bridges HAM's ~3.4 us busy window until the
    first xt+W tiles land.
  - fp8 was considered and rejected: max-rel-err would be ~5e-2 > 2e-2 gate.

Per core (slot-major):
  for slot s in [A(2 batches), B(1), C(1)]:
    for half (2 x 2048 cols):
      stream W[s]-half as 8 k-tiles [128, 2048] f16 on the sync HWDGE ring
      for m over the slot's 128-sample tiles (8 for A, 4 for B/C):
        for kt(8): 4 matmuls (2 psum tiles [128,1024] x 2 banks), accum kt
        evict psum + bias (2 DVE adds) -> out tile, DMA to out (scalar ring)
"""

import numpy as np

import concourse.bacc as bacc
import concourse.mybir as mybir
import concourse.bass as bass
import concourse.tile as tile
from concourse.bass_utils import run_bass_kernel_spmd

N_CORES = 8
B, S, K, H = 32, 512, 1024, 4096
BPC = B // N_CORES          # batches per core
P = 128                     # partitions
KT = K // P                 # k tiles (8)
MT = S // P                 # sample tiles per batch (4)
NHALF = 2                   # n halves
NH = H // NHALF             # cols per half (2048)
SLOT_BATCHES = (2, 1, 1)    # batches per weight slot
NSLOT = len(SLOT_BATCHES)
N_WARM = 20                 # warmup matmuls (N=256, round robin over 8 banks)

_COMPILED = None


def _build():
    nc = bacc.Bacc("TRN2", target_bir_lowering=False, debug=False)
    f32 = mybir.dt.float32
    f16 = mybir.dt.float16

    # xt: per batch, partition p holds x[b, :, kt*128+p] for kt=0..7, i.e.
    # row layout [kt, m] (8 KB contiguous per partition row).
    xt_ap = nc.dram_tensor("xt", [BPC, P, KT * S], f16, kind="ExternalInput").ap()
    # w: [slot, half, kt, p, n] so each (slot, half, kt) tile is [128, 2048]
    # with 4 KB contiguous per-partition rows.
    w_ap = nc.dram_tensor(
        "w", [NSLOT, NHALF, KT, P, NH], f16, kind="ExternalInput"
    ).ap()
    bias_ap = nc.dram_tensor("bias", [NSLOT, H], f16, kind="ExternalInput").ap()
    out_ap = nc.dram_tensor("out", [BPC, S, H], f32, kind="ExternalOutput").ap()

    with tile.TileContext(nc) as tc:
        with (
            tc.tile_pool(name="xt_pool", bufs=4) as xt_pool,
            tc.tile_pool(name="w_pool", bufs=16) as w_pool,
            tc.tile_pool(name="bias_pool", bufs=2) as bias_pool,
            tc.tile_pool(name="out_pool", bufs=4) as out_pool,
            tc.tile_pool(name="ps_pool", bufs=4, space="PSUM") as ps_pool,
        ):
            # Allocate xt and bias tiles up front. xt_b0 gates the very first
            # matmul, and the sync ring wakes ~4 us before the scalar ring,
            # so xt_b0 rides the sync ring ahead of all W. Everything not
            # needed in the first ~30 us is deferred via tile_wait_until so
            # the scheduler can't hoist it into the startup window.
            xt_ts = [
                xt_pool.tile([P, KT * S], f16, name="xt_t", tag="xt")
                for _ in range(BPC)
            ]
            bias_ts = [
                bias_pool.tile([P, H], f16, name="bias_t") for _ in range(NSLOT)
            ]

            def dma_xt(b, eng):
                eng.dma_start(xt_ts[b][:], xt_ap[b])

            def dma_bias(s):
                bias_src = bias_ap[s]
                nc.scalar.dma_start(
                    bias_ts[s][:],
                    bass.AP(
                        tensor=bias_src.tensor,
                        offset=bias_src.offset,
                        ap=[[0, P]] + list(bias_src.ap),
                    ),
                )

            # xt_b0 gates the first matmuls: split it in two 0.5 MB chunks
            # (kt 0-3, kt 4-7) so kt0's matmuls only wait for chunk 0.
            # Phase-0 W tiles (slot A, cols 0:1024) split across BOTH rings:
            # even kt on the scalar ring's front, odd kt on the sync ring
            # interleaved with the xt_b0 chunks. Everything else (bias, other
            # xt) is pushed out of the 0-20 us window via tile_wait_until.
            hx = KT * S // 2
            w_tiles_p0 = [
                w_pool.tile([P, 1024], f16, tag="w", name=f"w_p0_{kt}")
                for kt in range(KT)
            ]

            def dma_w_p0(kt, eng):
                eng.dma_start(w_tiles_p0[kt][:], w_ap[0, 0, kt, :, 0:1024])

            nc.sync.dma_start(xt_ts[0][:, 0:hx], xt_ap[0][:, 0:hx])
            dma_w_p0(0, nc.scalar)
            dma_w_p0(2, nc.scalar)
            nc.sync.dma_start(xt_ts[0][:, hx:], xt_ap[0][:, hx:])
            dma_w_p0(4, nc.scalar)
            dma_w_p0(6, nc.scalar)
            dma_w_p0(1, nc.sync)
            dma_w_p0(3, nc.sync)
            dma_w_p0(5, nc.sync)
            dma_w_p0(7, nc.sync)
            with tc.tile_wait_until(0.012):
                dma_bias(0)
            with tc.tile_wait_until(0.018):
                dma_xt(1, nc.scalar)
            with tc.tile_wait_until(0.030):
                dma_xt(2, nc.scalar)
            with tc.tile_wait_until(0.040):
                dma_xt(3, nc.scalar)
            with tc.tile_wait_until(0.050):
                dma_bias(1)
            with tc.tile_wait_until(0.070):
                dma_bias(2)

            # Warm up the PE (HAM un-throttle) while the first DMAs land:
            # short matmuls on memset tiles, round-robin over all 8 PSUM banks
            # so they pipeline at issue rate instead of serializing on one
            # bank. Results read once so DCE keeps them.
            warm_x = xt_pool.tile([P, P], f16, name="warm_x", tag="warm")
            warm_w = w_pool.tile([P, 256], f16, tag="warmw", name="warm_w")
            nc.vector.memset(warm_x[:], 0.0)
            nc.vector.memset(warm_w[:], 0.0)
            warm_ps = [
                ps_pool.tile([P, 1024], f32, tag="ps", name="warm_ps")
                for _ in range(4)
            ]
            for i in range(N_WARM):
                t = warm_ps[(i // 2) % 4]
                col = 512 * (i % 2)
                nc.tensor.matmul(
                    t[:, col : col + 256], warm_x[:], warm_w[:],
                    start=True, stop=True, skip_group_check=True,
                )
            warm_out = out_pool.tile([P, 16], f32, name="warm_out", tag="warmo")
            for i in range(4):
                nc.vector.tensor_copy(warm_out[:, 4 * i : 4 * i + 4], warm_ps[i][:, 0:4])

            # Phases: slot A (2 batches) runs 4 quarter-width (1024-col)
            # phases so the DMA-gated first m-sweep needs only 2 MB of W;
            # single-batch slots B/C run 2 half-width (2048-col) phases.
            phases = []
            bi0 = 0
            for s in range(NSLOT):
                nb = SLOT_BATCHES[s]
                ncols = 1024 if s == 0 else NH
                for col0 in range(0, H, ncols):
                    phases.append((s, bi0, nb, col0, ncols))
                bi0 += nb

            for pi, (s, bi0, nb, col0, ncols) in enumerate(phases):
                half, off = divmod(col0, NH)
                if pi == 0:
                    w_tiles = w_tiles_p0
                else:
                    w_tiles = []
                    for kt in range(KT):
                        w_t = w_pool.tile([P, ncols], f16, tag="w", name="w_t")
                        nc.sync.dma_start(
                            w_t[:], w_ap[s, half, kt, :, off : off + ncols]
                        )
                        w_tiles.append(w_t)
                nps = ncols // 1024
                for m in range(nb * MT):
                    b, mm = divmod(m, MT)
                    last_iter = pi == len(phases) - 1 and m == nb * MT - 1
                    ps = [
                        ps_pool.tile([P, 1024], f32, tag="ps", name="ps")
                        for _ in range(nps)
                    ]
                    if last_iter:
                        # n-major so ps0 finishes early: evict + store it
                        # while ps1's matmuls still run (shorter tail).
                        mm_order = [
                            (kt, n4) for n4 in range(2 * nps) for kt in range(KT)
                        ]
                    else:
                        mm_order = [
                            (kt, n4) for kt in range(KT) for n4 in range(2 * nps)
                        ]
                    for kt, n4 in mm_order:
                        lhsT = xt_ts[bi0 + b][
                            :, kt * S + mm * P : kt * S + (mm + 1) * P
                        ]
                        nc.tensor.matmul(
                            ps[n4 // 2][:, (n4 % 2) * 512 : (n4 % 2) * 512 + 512],
                            lhsT,
                            w_tiles[kt][:, n4 * 512 : (n4 + 1) * 512],
                            start=(kt == 0),
                            stop=(kt == KT - 1),
                        )
                    out_t = out_pool.tile([P, ncols], f32, tag="o", name="out_t")
                    if last_iter:
                        # Fine-grained (512-col) evict+store pipeline: each
                        # chunk's add and store start as soon as its bank's
                        # accumulation (n-major order) completes.
                        for q in range(2 * nps):
                            nc.vector.tensor_add(
                                out_t[:, q * 512 : (q + 1) * 512],
                                ps[q // 2][:, (q % 2) * 512 : (q % 2) * 512 + 512],
                                bias_ts[s][:, col0 + q * 512 : col0 + (q + 1) * 512],
                            )
                            nc.scalar.dma_start(
                                out_ap[
                                    bi0 + b,
                                    mm * P : (mm + 1) * P,
                                    col0 + q * 512 : col0 + (q + 1) * 512,
                                ],
                                out_t[:, q * 512 : (q + 1) * 512],
                            )
                    else:
                        for h2 in range(nps):
                            nc.vector.tensor_add(
                                out_t[:, h2 * 1024 : (h2 + 1) * 1024],
                                ps[h2][:],
                                bias_ts[s][:, col0 + h2 * 1024 : col0 + (h2 + 1) * 1024],
                            )
                        nc.scalar.dma_start(
                            out_ap[
                                bi0 + b,
                                mm * P : (mm + 1) * P,
                                col0 : col0 + ncols,
                            ],
                            out_t[:],
                        )
    nc.compile()
    return nc


def _get_compiled():
    global _COMPILED
    if _COMPILED is None:
        _COMPILED = _build()
    return _COMPILED


def _pack(cat_ids):
    """Assign batches to cores with slot capacities [2,1,1] per core.

    Returns per-core (idx, slot_cats): idx = 4 batch indices ordered
    [pair0, pair1, single_b, single_c]; slot_cats = categories for the 3 slots.
    Always feasible: #disjoint same-cat pairs = (32 - #odd-count cats)/2 >= 8.
    """
    cat_ids = np.asarray(cat_ids)
    by_cat = {}
    for i, c in enumerate(cat_ids.tolist()):
        by_cat.setdefault(c, []).append(i)
    pairs = []
    singles = []
    for c, idxs in sorted(by_cat.items()):
        n = len(idxs)
        for j in range(n // 2):
            pairs.append((c, idxs[2 * j], idxs[2 * j + 1]))
        if n % 2:
            singles.append((c, idxs[-1]))
    assert len(pairs) >= N_CORES, "impossible: <8 same-cat pairs among 32 batches"
    core_pairs = pairs[:N_CORES]
    # leftovers: extra pairs flatten into singles
    for c, i, j in pairs[N_CORES:]:
        singles.append((c, i))
        singles.append((c, j))
    assert len(singles) == 2 * N_CORES
    cores = []
    for ci in range(N_CORES):
        c, i, j = core_pairs[ci]
        (cb, ib), (cc, ic) = singles[2 * ci], singles[2 * ci + 1]
        cores.append(([i, j, ib, ic], [c, cb, cc]))
    return cores


def _host_pack_xt(xb):
    """x batches (n, 512, 1024) f32 -> (n, 128, KT*S) f16, kt-major rows.

    xt[b, p, kt*512 + m] = x[b, m, kt*128 + p]
    """
    n = xb.shape[0]
    xt = xb.astype(np.float16).transpose(0, 2, 1)          # (n, K, S)
    xt = xt.reshape(n, KT, P, S).transpose(0, 2, 1, 3)     # (n, P, KT, S)
    return np.ascontiguousarray(xt.reshape(n, P, KT * S))


def _host_pack_w(Wsel):
    """W slots (3, 1024, 4096) f32 -> (3, NHALF, KT, P, NH) f16.

    w[s, h, kt, p, j] = W[s, kt*128 + p, h*2048 + j]
    """
    w = Wsel.astype(np.float16).reshape(NSLOT, KT, P, NHALF, NH)
    return np.ascontiguousarray(w.transpose(0, 3, 1, 2, 4))


def run_sharded(x, cat_ids, W, b, trace=False, **spmd_kwargs):
    """Shard, run on 8 cores, unshard. Returns (out, BassKernelResults)."""
    x = np.ascontiguousarray(np.asarray(x), dtype=np.float32)
    cat_ids = np.asarray(cat_ids).astype(np.int64)
    W = np.ascontiguousarray(np.asarray(W), dtype=np.float32)
    b = np.ascontiguousarray(np.asarray(b), dtype=np.float32)

    nc = _get_compiled()
    cores = _pack(cat_ids)

    in_maps = []
    for idx, slot_cats in cores:
        in_maps.append(
            {
                "xt": _host_pack_xt(x[idx]),
                "w": _host_pack_w(W[slot_cats]),
                "bias": b[slot_cats].astype(np.float16),
            }
        )

    res = run_bass_kernel_spmd(
        nc, in_maps, list(range(N_CORES)), trace=trace, **spmd_kwargs
    )

    out = np.empty((B, S, H), dtype=np.float32)
    for c, (idx, _) in enumerate(cores):
        out[idx] = res.results[c]["out"]
    return out, res


def kernel(x, cat_ids, W, b):
    out, _ = run_sharded(x, cat_ids, W, b)
    return out


# revision 14
# speedup vs baseline: 1.0514x; 1.0202x over previous
"""Category-specific linear (MoE-style routed batched matmul) on 8 trn2 cores.

out[b, s, h] = sum_i x[b, s, i] * W[cat_ids[b], i, h] + bias[cat_ids[b], h]

Shapes (hardcoded): x (32, 512, 1024) f32, cat_ids (32,) int, W (16, 1024, 4096)
f32, b (16, 4096) f32 -> out (32, 512, 4096) f32.

Strategy: data-parallel over batch, 4 batches per core, with host-side routing
that always packs one same-category PAIR of batches plus two singles per core
(slot capacities [2, 1, 1] batches). With 32 batches over 16 categories there
are always >= (32 - 16)/2 = 8 disjoint same-category pairs, so this packing is
feasible for ANY cat_ids. Each core then loads only 3 weight matrices (24 MB
in f16) instead of 4, keeping the kernel compute-bound.

v2 schedule notes (from trace analysis of the v1 267 us baseline):
  - PE steady state was already at the 216 ns/MM issue-rate roofline; the
    ~45 us of slack was startup (12 us serialized warmup chain + W-ring
    starvation while the bias broadcast DMA burned ~150 GB/s) and tail.
  - bias now rides the scalar HWDGE ring in f16 (0.5 MB/slot, after xt),
    leaving the sync ring 100% for W from t=0; gpsimd SWDGE is unused.
  - xt is host-packed kt-major so each batch is one DMA with 8 KB
    contiguous per-partition rows (was 1 KB packets).
  - warmup is 20 short (N=256) matmuls round-robin over all 8 PSUM banks
    (pipelined, ~220 ns each warm / 430 cold) instead of 22 serialized
    same-bank matmuls; it just bridges HAM's ~3.4 us busy window until the
    first xt+W tiles land.
  - fp8 was considered and rejected: max-rel-err would be ~5e-2 > 2e-2 gate.

Per core (slot-major):
  for slot s in [A(2 batches), B(1), C(1)]:
    for half (2 x 2048 cols):
      stream W[s]-half as 8 k-tiles [128, 2048] f16 on the sync HWDGE ring
      for m over the slot's 128-sample tiles (8 for A, 4 for B/C):
        for kt(8): 4 matmuls (2 psum tiles [128,1024] x 2 banks), accum kt
        evict psum + bias (2 DVE adds) -> out tile, DMA to out (scalar ring)
"""

import numpy as np

import concourse.bacc as bacc
import concourse.mybir as mybir
import concourse.bass as bass
import concourse.tile as tile
from concourse.bass_utils import run_bass_kernel_spmd

N_CORES = 8
B, S, K, H = 32, 512, 1024, 4096
BPC = B // N_CORES          # batches per core
P = 128                     # partitions
KT = K // P                 # k tiles (8)
MT = S // P                 # sample tiles per batch (4)
NHALF = 2                   # n halves
NH = H // NHALF             # cols per half (2048)
SLOT_BATCHES = (2, 1, 1)    # batches per weight slot
NSLOT = len(SLOT_BATCHES)
N_WARM = 20                 # warmup matmuls (N=256, round robin over 8 banks)

_COMPILED = None


def _build():
    nc = bacc.Bacc("TRN2", target_bir_lowering=False, debug=False)
    f32 = mybir.dt.float32
    f16 = mybir.dt.float16

    # xt: per batch, partition p holds x[b, :, kt*128+p] for kt=0..7, i.e.
    # row layout [kt, m] (8 KB contiguous per partition row).
    xt_ap = nc.dram_tensor("xt", [BPC, P, KT * S], f16, kind="ExternalInput").ap()
    # w: [slot, half, kt, p, n] so each (slot, half, kt) tile is [128, 2048]
    # with 4 KB contiguous per-partition rows.
    w_ap = nc.dram_tensor(
        "w", [NSLOT, NHALF, KT, P, NH], f16, kind="ExternalInput"
    ).ap()
    bias_ap = nc.dram_tensor("bias", [NSLOT, H], f16, kind="ExternalInput").ap()
    out_ap = nc.dram_tensor("out", [BPC, S, H], f32, kind="ExternalOutput").ap()

    with tile.TileContext(nc) as tc:
        with (
            tc.tile_pool(name="xt_pool", bufs=4) as xt_pool,
            tc.tile_pool(name="w_pool", bufs=16) as w_pool,
            tc.tile_pool(name="bias_pool", bufs=2) as bias_pool,
            tc.tile_pool(name="out_pool", bufs=4) as out_pool,
            tc.tile_pool(name="ps_pool", bufs=4, space="PSUM") as ps_pool,
        ):
            # Allocate xt and bias tiles up front. xt_b0 gates the very first
            # matmul, and the sync ring wakes ~4 us before the scalar ring,
            # so xt_b0 rides the sync ring ahead of all W. Everything not
            # needed in the first ~30 us is deferred via tile_wait_until so
            # the scheduler can't hoist it into the startup window.
            xt_ts = [
                xt_pool.tile([P, KT * S], f16, name="xt_t", tag="xt")
                for _ in range(BPC)
            ]
            bias_ts = [
                bias_pool.tile([P, H], f16, name="bias_t") for _ in range(NSLOT)
            ]

            def dma_xt(b, eng):
                eng.dma_start(xt_ts[b][:], xt_ap[b])

            def dma_bias(s, eng):
                bias_src = bias_ap[s]
                eng.dma_start(
                    bias_ts[s][:],
                    bass.AP(
                        tensor=bias_src.tensor,
                        offset=bias_src.offset,
                        ap=[[0, P]] + list(bias_src.ap),
                    ),
                )

            # xt_b0 gates the first matmuls: split it in two 0.5 MB chunks
            # (kt 0-3, kt 4-7) so kt0's matmuls only wait for chunk 0.
            # Phase-0 W tiles (slot A, cols 0:1024) split across BOTH rings:
            # even kt on the scalar ring's front, odd kt on the sync ring
            # interleaved with the xt_b0 chunks. Everything else (bias, other
            # xt) is pushed out of the 0-20 us window via tile_wait_until.
            hx = KT * S // 2
            w_tiles_p0 = [
                w_pool.tile([P, 1024], f16, tag="w", name=f"w_p0_{kt}")
                for kt in range(KT)
            ]

            def dma_w_p0(kt, eng):
                eng.dma_start(w_tiles_p0[kt][:], w_ap[0, 0, kt, :, 0:1024])

            nc.sync.dma_start(xt_ts[0][:, 0:hx], xt_ap[0][:, 0:hx])
            dma_w_p0(0, nc.scalar)
            dma_w_p0(2, nc.scalar)
            nc.sync.dma_start(xt_ts[0][:, hx:], xt_ap[0][:, hx:])
            dma_w_p0(4, nc.scalar)
            dma_w_p0(6, nc.scalar)
            dma_w_p0(1, nc.sync)
            dma_w_p0(3, nc.sync)
            dma_w_p0(5, nc.sync)
            dma_w_p0(7, nc.sync)
            # bias_A rides the sync ring right behind phase-0's W (~15 us,
            # well before the first eviction's psum-recycle deadline).
            dma_bias(0, nc.sync)
            with tc.tile_wait_until(0.012):
                dma_xt(1, nc.scalar)
            with tc.tile_wait_until(0.030):
                dma_xt(2, nc.scalar)
            with tc.tile_wait_until(0.040):
                dma_xt(3, nc.scalar)
            with tc.tile_wait_until(0.050):
                dma_bias(1, nc.scalar)
            with tc.tile_wait_until(0.070):
                dma_bias(2, nc.scalar)

            # Warm up the PE (HAM un-throttle) while the first DMAs land:
            # short matmuls on memset tiles, round-robin over all 8 PSUM banks
            # so they pipeline at issue rate instead of serializing on one
            # bank. Results read once so DCE keeps them.
            warm_x = xt_pool.tile([P, P], f16, name="warm_x", tag="warm")
            warm_w = w_pool.tile([P, 256], f16, tag="warmw", name="warm_w")
            nc.vector.memset(warm_x[:], 0.0)
            nc.vector.memset(warm_w[:], 0.0)
            warm_ps = [
                ps_pool.tile([P, 1024], f32, tag="ps", name="warm_ps")
                for _ in range(4)
            ]
            for i in range(N_WARM):
                t = warm_ps[(i // 2) % 4]
                col = 512 * (i % 2)
                nc.tensor.matmul(
                    t[:, col : col + 256], warm_x[:], warm_w[:],
                    start=True, stop=True, skip_group_check=True,
                )
            warm_out = out_pool.tile([P, 16], f32, name="warm_out", tag="warmo")
            for i in range(4):
                nc.vector.tensor_copy(warm_out[:, 4 * i : 4 * i + 4], warm_ps[i][:, 0:4])

            # Phases: slot A (2 batches) runs 4 quarter-width (1024-col)
            # phases so the DMA-gated first m-sweep needs only 2 MB of W;
            # single-batch slots B/C run 2 half-width (2048-col) phases.
            phases = []
            bi0 = 0
            for s in range(NSLOT):
                nb = SLOT_BATCHES[s]
                ncols = 1024 if s == 0 else NH
                for col0 in range(0, H, ncols):
                    phases.append((s, bi0, nb, col0, ncols))
                bi0 += nb

            for pi, (s, bi0, nb, col0, ncols) in enumerate(phases):
                half, off = divmod(col0, NH)
                if pi == 0:
                    w_tiles = w_tiles_p0
                else:
                    w_tiles = []
                    for kt in range(KT):
                        w_t = w_pool.tile([P, ncols], f16, tag="w", name="w_t")
                        nc.sync.dma_start(
                            w_t[:], w_ap[s, half, kt, :, off : off + ncols]
                        )
                        w_tiles.append(w_t)
                nps = ncols // 1024
                for m in range(nb * MT):
                    b, mm = divmod(m, MT)
                    last_iter = pi == len(phases) - 1 and m == nb * MT - 1
                    ps = [
                        ps_pool.tile([P, 1024], f32, tag="ps", name="ps")
                        for _ in range(nps)
                    ]
                    if last_iter:
                        # n-major so ps0 finishes early: evict + store it
                        # while ps1's matmuls still run (shorter tail).
                        mm_order = [
                            (kt, n4) for n4 in range(2 * nps) for kt in range(KT)
                        ]
                    else:
                        mm_order = [
                            (kt, n4) for kt in range(KT) for n4 in range(2 * nps)
                        ]
                    for kt, n4 in mm_order:
                        lhsT = xt_ts[bi0 + b][
                            :, kt * S + mm * P : kt * S + (mm + 1) * P
                        ]
                        nc.tensor.matmul(
                            ps[n4 // 2][:, (n4 % 2) * 512 : (n4 % 2) * 512 + 512],
                            lhsT,
                            w_tiles[kt][:, n4 * 512 : (n4 + 1) * 512],
                            start=(kt == 0),
                            stop=(kt == KT - 1),
                        )
                    out_t = out_pool.tile([P, ncols], f32, tag="o", name="out_t")
                    if last_iter:
                        # Fine-grained (512-col) evict+store pipeline: each
                        # chunk's add and store start as soon as its bank's
                        # accumulation (n-major order) completes.
                        for q in range(2 * nps):
                            nc.vector.tensor_add(
                                out_t[:, q * 512 : (q + 1) * 512],
                                ps[q // 2][:, (q % 2) * 512 : (q % 2) * 512 + 512],
                                bias_ts[s][:, col0 + q * 512 : col0 + (q + 1) * 512],
                            )
                            nc.scalar.dma_start(
                                out_ap[
                                    bi0 + b,
                                    mm * P : (mm + 1) * P,
                                    col0 + q * 512 : col0 + (q + 1) * 512,
                                ],
                                out_t[:, q * 512 : (q + 1) * 512],
                            )
                    else:
                        for h2 in range(nps):
                            nc.vector.tensor_add(
                                out_t[:, h2 * 1024 : (h2 + 1) * 1024],
                                ps[h2][:],
                                bias_ts[s][:, col0 + h2 * 1024 : col0 + (h2 + 1) * 1024],
                            )
                        nc.scalar.dma_start(
                            out_ap[
                                bi0 + b,
                                mm * P : (mm + 1) * P,
                                col0 : col0 + ncols,
                            ],
                            out_t[:],
                        )
    nc.compile()
    return nc


def _get_compiled():
    global _COMPILED
    if _COMPILED is None:
        _COMPILED = _build()
    return _COMPILED


def _pack(cat_ids):
    """Assign batches to cores with slot capacities [2,1,1] per core.

    Returns per-core (idx, slot_cats): idx = 4 batch indices ordered
    [pair0, pair1, single_b, single_c]; slot_cats = categories for the 3 slots.
    Always feasible: #disjoint same-cat pairs = (32 - #odd-count cats)/2 >= 8.
    """
    cat_ids = np.asarray(cat_ids)
    by_cat = {}
    for i, c in enumerate(cat_ids.tolist()):
        by_cat.setdefault(c, []).append(i)
    pairs = []
    singles = []
    for c, idxs in sorted(by_cat.items()):
        n = len(idxs)
        for j in range(n // 2):
            pairs.append((c, idxs[2 * j], idxs[2 * j + 1]))
        if n % 2:
            singles.append((c, idxs[-1]))
    assert len(pairs) >= N_CORES, "impossible: <8 same-cat pairs among 32 batches"
    core_pairs = pairs[:N_CORES]
    # leftovers: extra pairs flatten into singles
    for c, i, j in pairs[N_CORES:]:
        singles.append((c, i))
        singles.append((c, j))
    assert len(singles) == 2 * N_CORES
    cores = []
    for ci in range(N_CORES):
        c, i, j = core_pairs[ci]
        (cb, ib), (cc, ic) = singles[2 * ci], singles[2 * ci + 1]
        cores.append(([i, j, ib, ic], [c, cb, cc]))
    return cores


def _host_pack_xt(xb):
    """x batches (n, 512, 1024) f32 -> (n, 128, KT*S) f16, kt-major rows.

    xt[b, p, kt*512 + m] = x[b, m, kt*128 + p]
    """
    n = xb.shape[0]
    xt = xb.astype(np.float16).transpose(0, 2, 1)          # (n, K, S)
    xt = xt.reshape(n, KT, P, S).transpose(0, 2, 1, 3)     # (n, P, KT, S)
    return np.ascontiguousarray(xt.reshape(n, P, KT * S))


def _host_pack_w(Wsel):
    """W slots (3, 1024, 4096) f32 -> (3, NHALF, KT, P, NH) f16.

    w[s, h, kt, p, j] = W[s, kt*128 + p, h*2048 + j]
    """
    w = Wsel.astype(np.float16).reshape(NSLOT, KT, P, NHALF, NH)
    return np.ascontiguousarray(w.transpose(0, 3, 1, 2, 4))


def run_sharded(x, cat_ids, W, b, trace=False, **spmd_kwargs):
    """Shard, run on 8 cores, unshard. Returns (out, BassKernelResults)."""
    x = np.ascontiguousarray(np.asarray(x), dtype=np.float32)
    cat_ids = np.asarray(cat_ids).astype(np.int64)
    W = np.ascontiguousarray(np.asarray(W), dtype=np.float32)
    b = np.ascontiguousarray(np.asarray(b), dtype=np.float32)

    nc = _get_compiled()
    cores = _pack(cat_ids)

    in_maps = []
    for idx, slot_cats in cores:
        in_maps.append(
            {
                "xt": _host_pack_xt(x[idx]),
                "w": _host_pack_w(W[slot_cats]),
                "bias": b[slot_cats].astype(np.float16),
            }
        )

    res = run_bass_kernel_spmd(
        nc, in_maps, list(range(N_CORES)), trace=trace, **spmd_kwargs
    )

    out = np.empty((B, S, H), dtype=np.float32)
    for c, (idx, _) in enumerate(cores):
        out[idx] = res.results[c]["out"]
    return out, res


def kernel(x, cat_ids, W, b):
    out, _ = run_sharded(x, cat_ids, W, b)
    return out
